# revision 20
# baseline (speedup 1.0000x reference)
"""Trainium2 Bass kernel for the seasonal-decomposition block.

Math: for each season s, circ_s = real(F_s^H diag(d_s) F_s) with F_s the s-th
diagonal LxL block of the normalized N=L*S DFT matrix. Expanding,
    circ_s[a, b] = (1/N) * sum_j d_s[j] * cos(2*pi*(s*L+j)*(a-b)/N)
depends only on a-b: a symmetric Toeplitz matrix whose first column
c_s(t) is computed on host with one length-N FFT. Every 128x128 block of
circ_s is a contiguous column slice of the skewed buffer
    E2r_s[p, m] = c_s(|2047 + p - m|)   (shape [128, 4096], 2 MB fp32)
so the LxL matrix is never materialized; the PE reads stationary operands
straight out of an 8 MB SBUF-resident E2r.

The recurrence  x_rem <- x_rem - tanh(x_rem @ circ_s)  runs in transposed
layout (positions on partitions, rows on the free axis) so no per-season
transposes are needed:  out[b, r] = sum_a circ[a, b] * xT[a, r]  via
matmul(lhsT=circ_block, rhs=xT_chunk).  The trailing avg-pool trend is two
banded matmuls per 128-chunk with three tiny host-built band matrices.
The output is accumulated as sum_s tanh_s + trend (never x - x_rem, which
would lose precision to cancellation).

Sharding: pure data-parallel over the B*C = 2048 rows, 256 rows per core,
8 cores, no collectives. Matmuls run in float32r (full PE rate, ~1.6e-4
relative error) with fp32 PSUM accumulation.
"""

import sys

sys.path.insert(0, "/opt/trn_rl_repo")

import numpy as np

import concourse.mybir as mybir
import concourse.tile as tile
from concourse import bacc
from concourse.bass_utils import run_bass_kernel_spmd

L = 2048
S = 4
NFULL = L * S
KER = 25
B, C = 64, 32
NCORES = 8
ROWS = B * C          # 2048
RPC = ROWS // NCORES  # 256 rows per core
NCHUNK = L // 128     # 16

_f32 = mybir.dt.float32
_f32r = mybir.dt.float32r


def _build_tband():
    """Three [128,128] band blocks of the avg-pool matrix T (trend = T.T @ x)."""
    u = np.arange(128)[:, None]
    t = np.arange(128)[None, :]
    diag = ((t - u >= 0) & (t - u <= KER - 1)).astype(np.float32) / KER
    sub = ((u - t) >= 128 - (KER - 1)).astype(np.float32) / KER
    t00 = diag.copy()
    t00[0, :] += np.maximum(0, (KER - 1) - np.arange(128)).astype(np.float32) / KER
    return np.ascontiguousarray(np.stack([t00, diag, sub], axis=1))  # [128, 3, 128]


_TBAND = _build_tband()
_E2R_IDX = np.clip(np.abs(2047 + np.arange(128)[:, None] - np.arange(4096)[None, :]), 0, L - 1)


def _circ_cols(diagonals):
    """First columns c_s(t), t = 0..L-1, of each season's Toeplitz circ_s."""
    d = np.zeros((S, NFULL), dtype=np.float64)
    d[:, :L] = np.asarray(diagonals, dtype=np.float64)
    F = np.fft.fft(d, axis=1)  # F[s,k] = sum_j d_j e^{-2pi i jk/N}
    t = np.arange(L)
    ph = np.exp((2j * np.pi / NFULL) * (np.arange(S)[:, None] * L * t[None, :]))
    return ((ph * np.conj(F[:, :L])).real / NFULL).astype(np.float32)  # [S, L]


def _emit_body(nc, pools, xr_d, e2l_d, e2rr_d, tb_d, out_d, mmdt=_f32r):
    constp, xrp, corrp, workp, psum_a, psum_t = pools
    tanh_f = mybir.ActivationFunctionType.Tanh

    # Prologue DMA order follows first use: the opening accumulation chain
    # needs x quarter 0 + the hi half of season-0 weights, then consumes x
    # quarters and weight pieces alternately.
    x0t = [constp.tile([128, 4, RPC], mmdt, tag=f"x0_{k}", name=f"x0_{k}") for k in range(4)]
    e2lh_sb, e2ll_sb, e2rr_sb = [], [], []
    for s in range(S):
        e2lh_sb.append(constp.tile([128, 1024], mmdt, tag=f"e2lh{s}", name=f"e2lh{s}"))
        e2ll_sb.append(constp.tile([128, 1024], mmdt, tag=f"e2ll{s}", name=f"e2ll{s}"))
        e2rr_sb.append(constp.tile([128, 1920], mmdt, tag=f"e2rr{s}", name=f"e2rr{s}"))
    nc.sync.dma_start(x0t[0][:], xr_d[0])
    nc.gpsimd.dma_start(e2lh_sb[0][:], e2l_d[0][:, 1024:])
    nc.sync.dma_start(x0t[1][:], xr_d[1])
    nc.gpsimd.dma_start(e2ll_sb[0][:], e2l_d[0][:, :1024])
    nc.sync.dma_start(x0t[2][:], xr_d[2])
    nc.sync.dma_start(x0t[3][:], xr_d[3])
    nc.gpsimd.dma_start(e2rr_sb[0][:], e2rr_d[0])
    for s in range(1, S):
        nc.gpsimd.dma_start(e2lh_sb[s][:], e2l_d[s][:, 1024:])
        nc.gpsimd.dma_start(e2ll_sb[s][:], e2l_d[s][:, :1024])
        nc.gpsimd.dma_start(e2rr_sb[s][:], e2rr_d[s])
    xr_cur = [x0t[a // 4][:, a % 4, :] for a in range(NCHUNK)]
    tb_sb = constp.tile([128, 3, 128], mmdt, tag="tb")
    nc.sync.dma_start(tb_sb[:], tb_d[:])

    def circ_block(s, a, b):
        d = a - b
        if 0 <= d <= 7:
            return e2lh_sb[s][:, 896 - 128 * d : 1024 - 128 * d]
        if d >= 8:
            return e2ll_sb[s][:, 1920 - 128 * d : 2048 - 128 * d]
        return e2rr_sb[s][:, -128 * (d + 1) : -128 * d]

    corr = [corrp.tile([128, RPC], _f32, tag=f"corr{b}", name=f"corr{b}") for b in range(NCHUNK)]
    big_ob = constp.tile([128, NCHUNK, RPC], _f32, tag="bigob")

    def emit_trend(j, xr3):
        tps = psum_t.tile([128, RPC], _f32, tag="acc" if psum_t is psum_a else "tps", name=f"tps{j}")
        if j == 0:
            nc.tensor.matmul(tps[:], tb_sb[:, 0, :], xr3[0], start=True, stop=True)
        else:
            nc.tensor.matmul(tps[:], tb_sb[:, 2, :], xr3[j - 1], start=True, stop=False)
            nc.tensor.matmul(tps[:], tb_sb[:, 1, :], xr3[j], start=False, stop=True)
        nc.vector.tensor_add(out=big_ob[:, j, :], in0=corr[j][:], in1=tps[:])
        if j % 4 == 3:
            q = j // 4
            nc.sync.dma_start(out_d[:, 4 * q : 4 * q + 4, :], big_ob[:, 4 * q : 4 * q + 4, :])

    for s in range(S):
        xr_next = [xrp.tile([128, RPC], mmdt, tag=f"xr{b}", name=f"xr{s}_{b}") for b in range(NCHUNK)]
        for b in range(NCHUNK):
            acc = psum_a.tile([128, RPC], _f32, tag="acc")
            # Chain order a = b..15 then 0..b-1: the a >= b blocks live in the
            # left weight halves, which arrive first; the a < b blocks (right
            # half) come last so season 0 never stalls on the e2rr DMA.
            a_order = list(range(b, NCHUNK)) + list(range(b))
            for i, a in enumerate(a_order):
                nc.tensor.matmul(
                    acc[:],
                    circ_block(s, a, b),
                    xr_cur[a],
                    start=(i == 0),
                    stop=(i == NCHUNK - 1),
                )
            if s == 0:
                nc.scalar.activation(corr[b][:], acc[:], tanh_f)
                nc.vector.tensor_sub(out=xr_next[b][:], in0=xr_cur[b], in1=corr[b][:])
            else:
                tmp = workp.tile([128, RPC], _f32, tag="tanh")
                nc.scalar.activation(tmp[:], acc[:], tanh_f)
                nc.vector.tensor_add(out=corr[b][:], in0=corr[b][:], in1=tmp[:])
                nc.vector.tensor_sub(out=xr_next[b][:], in0=xr_cur[b], in1=tmp[:])
            # Interleave trend chunks two groups behind season 3 so the PE
            # never waits on the DVE updates they read.
            if s == S - 1 and b >= 2:
                emit_trend(b - 2, xr_next)
        xr_cur = [t_[:] for t_ in xr_next]

    emit_trend(NCHUNK - 2, xr_cur)
    emit_trend(NCHUNK - 1, xr_cur)


def build_nc(reps=1, acc_bufs=6, merge_tps=True, mmdt=_f32r):
    nc = bacc.Bacc("TRN2", target_bir_lowering=False, debug=False)
    xr_d = nc.dram_tensor("xr", [4, 128, 4, RPC], mmdt, kind="ExternalInput")
    e2l_d = nc.dram_tensor("e2l", [S, 128, 2048], mmdt, kind="ExternalInput")
    e2rr_d = nc.dram_tensor("e2rr", [S, 128, 1920], mmdt, kind="ExternalInput")
    tb_d = nc.dram_tensor("tb", [128, 3, 128], mmdt, kind="ExternalInput")
    out_d = nc.dram_tensor("out", [128, NCHUNK, RPC], _f32, kind="ExternalOutput")

    with tile.TileContext(nc) as tc:
        with (
            tc.tile_pool(name="const", bufs=1) as constp,
            tc.tile_pool(name="xrp", bufs=2) as xrp,
            tc.tile_pool(name="corrp", bufs=1) as corrp,
            tc.tile_pool(name="work", bufs=4) as workp,
            tc.tile_pool(name="psum_a", bufs=acc_bufs, space="PSUM") as psum_a,
            tc.tile_pool(name="psum_t", bufs=2, space="PSUM") as psum_t,
        ):
            pools = (constp, xrp, corrp, workp, psum_a,
                     psum_a if merge_tps else psum_t)
            if reps == 1:
                _emit_body(nc, pools, xr_d, e2l_d, e2rr_d, tb_d, out_d, mmdt)
            else:
                with tc.For_i(0, reps, 1, staggered_reset=True,
                              hint_engines=(mybir.EngineType.PE,)):
                    _emit_body(nc, pools, xr_d, e2l_d, e2rr_d, tb_d, out_d, mmdt)
    nc.compile()
    return nc


_NC_CACHE = {}


def _get_nc(reps=1):
    if reps not in _NC_CACHE:
        _NC_CACHE[reps] = build_nc(reps)
    return _NC_CACHE[reps]


def make_in_maps(x, diagonals, np_dt=np.float32):
    c = _circ_cols(diagonals)
    e2r = c[:, _E2R_IDX]  # [S, 128, 4096]
    e2l = np.ascontiguousarray(e2r[:, :, 127:2175]).astype(np_dt)
    e2rr = np.ascontiguousarray(e2r[:, :, 2175:4095]).astype(np_dt)
    xT = np.asarray(x, dtype=np.float32).reshape(ROWS, L).T  # [L, ROWS] view
    in_maps = []
    for i in range(NCORES):
        xs = np.ascontiguousarray(xT[:, i * RPC : (i + 1) * RPC])
        xs = xs.reshape(NCHUNK, 128, RPC).transpose(1, 0, 2)  # [128, 16, RPC]
        xs = np.ascontiguousarray(xs.reshape(128, 4, 4, RPC).transpose(1, 0, 2, 3))
        in_maps.append({"xr": xs.astype(np_dt), "e2l": e2l, "e2rr": e2rr, "tb": _TBAND.astype(np_dt)})
    return in_maps


def gather_out(results):
    parts = []
    for r in results:
        o = r["out"]  # [128, NCHUNK, RPC]
        parts.append(np.ascontiguousarray(o.transpose(1, 0, 2)).reshape(L, RPC))
    outT = np.concatenate(parts, axis=1)  # [L, ROWS]
    return np.ascontiguousarray(outT.T).reshape(B, C, L).astype(np.float32)


def kernel(x, diagonals):
    x = np.asarray(x, dtype=np.float32)
    assert x.shape == (B, C, L) and np.asarray(diagonals).shape == (S, L)
    nc = _get_nc(1)
    in_maps = make_in_maps(x, diagonals)
    last_err = None
    for attempt in range(3):
        try:
            res = run_bass_kernel_spmd(nc, in_maps, core_ids=list(range(NCORES)))
            return gather_out(res.results)
        except Exception as ex:  # transient device errors (e.g. NRT_EXEC_UNIT_UNRECOVERABLE)
            last_err = ex
            import time as _time

            _time.sleep(2.0 * (attempt + 1))
    raise last_err


# revision 21
# speedup vs baseline: 1.2366x; 1.2366x over previous
"""Trainium2 Bass kernel for the seasonal-decomposition block.

Math: for each season s, circ_s = real(F_s^H diag(d_s) F_s) with F_s the s-th
diagonal LxL block of the normalized N=L*S DFT matrix. Expanding,
    circ_s[a, b] = (1/N) * sum_j d_s[j] * cos(2*pi*(s*L+j)*(a-b)/N)
depends only on a-b: a symmetric Toeplitz matrix whose first column
c_s(t) is computed on host with one length-N FFT. Every 128x128 block of
circ_s is a contiguous column slice of the skewed buffer
    E2r_s[p, m] = c_s(|2047 + p - m|)   (shape [128, 4096], 2 MB fp32)
so the LxL matrix is never materialized; the PE reads stationary operands
straight out of an 8 MB SBUF-resident E2r.

The recurrence  x_rem <- x_rem - tanh(x_rem @ circ_s)  runs in transposed
layout (positions on partitions, rows on the free axis) so no per-season
transposes are needed:  out[b, r] = sum_a circ[a, b] * xT[a, r]  via
matmul(lhsT=circ_block, rhs=xT_chunk).  The trailing avg-pool trend is two
banded matmuls per 128-chunk with three tiny host-built band matrices.
The output is accumulated as sum_s tanh_s + trend (never x - x_rem, which
would lose precision to cancellation).

Sharding: pure data-parallel over the B*C = 2048 rows, 256 rows per core,
8 cores, no collectives. Matmuls run in float32r (full PE rate, ~1.6e-4
relative error) with fp32 PSUM accumulation.
"""

import sys

sys.path.insert(0, "/opt/trn_rl_repo")

import numpy as np

import concourse.mybir as mybir
import concourse.tile as tile
from concourse import bacc
from concourse.bass_utils import run_bass_kernel_spmd

L = 2048
S = 4
NFULL = L * S
KER = 25
B, C = 64, 32
NCORES = 8
ROWS = B * C          # 2048
RPC = ROWS // NCORES  # 256 rows per core
NCHUNK = L // 128     # 16

_f32 = mybir.dt.float32
_f32r = mybir.dt.float32r


def _build_tband():
    """Three [128,128] band blocks of the avg-pool matrix T (trend = T.T @ x)."""
    u = np.arange(128)[:, None]
    t = np.arange(128)[None, :]
    diag = ((t - u >= 0) & (t - u <= KER - 1)).astype(np.float32) / KER
    sub = ((u - t) >= 128 - (KER - 1)).astype(np.float32) / KER
    t00 = diag.copy()
    t00[0, :] += np.maximum(0, (KER - 1) - np.arange(128)).astype(np.float32) / KER
    return np.ascontiguousarray(np.stack([t00, diag, sub], axis=1))  # [128, 3, 128]


_TBAND = _build_tband()
_E2R_IDX = np.clip(np.abs(2047 + np.arange(128)[:, None] - np.arange(4096)[None, :]), 0, L - 1)


def _circ_cols(diagonals):
    """First columns c_s(t), t = 0..L-1, of each season's Toeplitz circ_s."""
    d = np.zeros((S, NFULL), dtype=np.float64)
    d[:, :L] = np.asarray(diagonals, dtype=np.float64)
    F = np.fft.fft(d, axis=1)  # F[s,k] = sum_j d_j e^{-2pi i jk/N}
    t = np.arange(L)
    ph = np.exp((2j * np.pi / NFULL) * (np.arange(S)[:, None] * L * t[None, :]))
    return ((ph * np.conj(F[:, :L])).real / NFULL).astype(np.float32)  # [S, L]


def _emit_body(nc, pools, xr_d, e2l_d, e2rr_d, tb_d, out_d, mmdt=_f32r):
    constp, xrp, corrp, workp, psum_a, psum_t = pools
    tanh_f = mybir.ActivationFunctionType.Tanh

    # Prologue DMA order follows first use: the opening accumulation chain
    # needs x quarter 0 + the hi half of season-0 weights, then consumes x
    # quarters and weight pieces alternately.
    x0t = [constp.tile([128, 4, RPC], mmdt, tag=f"x0_{k}", name=f"x0_{k}") for k in range(4)]
    e2lh_sb, e2ll_sb, e2rr_sb = [], [], []
    for s in range(S):
        e2lh_sb.append(constp.tile([128, 1024], mmdt, tag=f"e2lh{s}", name=f"e2lh{s}"))
        e2ll_sb.append(constp.tile([128, 1024], mmdt, tag=f"e2ll{s}", name=f"e2ll{s}"))
        e2rr_sb.append(constp.tile([128, 1920], mmdt, tag=f"e2rr{s}", name=f"e2rr{s}"))
    nc.sync.dma_start(x0t[0][:], xr_d[0])
    nc.sync.dma_start(e2lh_sb[0][:], e2l_d[0][:, 1024:])
    nc.sync.dma_start(x0t[1][:], xr_d[1])
    nc.sync.dma_start(e2ll_sb[0][:], e2l_d[0][:, :1024])
    nc.sync.dma_start(x0t[2][:], xr_d[2])
    nc.sync.dma_start(x0t[3][:], xr_d[3])
    nc.sync.dma_start(e2rr_sb[0][:], e2rr_d[0])
    for s in range(1, S):
        nc.sync.dma_start(e2lh_sb[s][:], e2l_d[s][:, 1024:])
        nc.sync.dma_start(e2ll_sb[s][:], e2l_d[s][:, :1024])
        nc.sync.dma_start(e2rr_sb[s][:], e2rr_d[s])
    xr_cur = [x0t[a // 4][:, a % 4, :] for a in range(NCHUNK)]
    tb_sb = constp.tile([128, 3, 128], mmdt, tag="tb")
    nc.sync.dma_start(tb_sb[:], tb_d[:])

    def circ_block(s, a, b):
        d = a - b
        if 0 <= d <= 7:
            return e2lh_sb[s][:, 896 - 128 * d : 1024 - 128 * d]
        if d >= 8:
            return e2ll_sb[s][:, 1920 - 128 * d : 2048 - 128 * d]
        return e2rr_sb[s][:, -128 * (d + 1) : -128 * d]

    corr = [corrp.tile([128, RPC], _f32, tag=f"corr{b}", name=f"corr{b}") for b in range(NCHUNK)]
    big_ob = constp.tile([128, NCHUNK, RPC], _f32, tag="bigob")

    def emit_trend(j, xr3):
        tps = psum_t.tile([128, RPC], _f32, tag="acc" if psum_t is psum_a else "tps", name=f"tps{j}")
        if j == 0:
            nc.tensor.matmul(tps[:], tb_sb[:, 0, :], xr3[0], start=True, stop=True)
        else:
            nc.tensor.matmul(tps[:], tb_sb[:, 2, :], xr3[j - 1], start=True, stop=False)
            nc.tensor.matmul(tps[:], tb_sb[:, 1, :], xr3[j], start=False, stop=True)
        nc.vector.tensor_add(out=big_ob[:, j, :], in0=corr[j][:], in1=tps[:])
        if j % 4 == 3:
            q = j // 4
            nc.sync.dma_start(out_d[:, 4 * q : 4 * q + 4, :], big_ob[:, 4 * q : 4 * q + 4, :])

    for s in range(S):
        xr_next = [xrp.tile([128, RPC], mmdt, tag=f"xr{b}", name=f"xr{s}_{b}") for b in range(NCHUNK)]
        for b in range(NCHUNK):
            acc = psum_a.tile([128, RPC], _f32, tag="acc")
            # Chain order a = b..15 then 0..b-1: the a >= b blocks live in the
            # left weight halves, which arrive first; the a < b blocks (right
            # half) come last so season 0 never stalls on the e2rr DMA.
            a_order = list(range(b, NCHUNK)) + list(range(b))
            for i, a in enumerate(a_order):
                nc.tensor.matmul(
                    acc[:],
                    circ_block(s, a, b),
                    xr_cur[a],
                    start=(i == 0),
                    stop=(i == NCHUNK - 1),
                )
            if s == 0:
                nc.scalar.activation(corr[b][:], acc[:], tanh_f)
                nc.vector.tensor_sub(out=xr_next[b][:], in0=xr_cur[b], in1=corr[b][:])
            else:
                tmp = workp.tile([128, RPC], _f32, tag="tanh")
                nc.scalar.activation(tmp[:], acc[:], tanh_f)
                nc.vector.tensor_add(out=corr[b][:], in0=corr[b][:], in1=tmp[:])
                nc.vector.tensor_sub(out=xr_next[b][:], in0=xr_cur[b], in1=tmp[:])
            # Interleave trend chunks two groups behind season 3 so the PE
            # never waits on the DVE updates they read.
            if s == S - 1 and b >= 2:
                emit_trend(b - 2, xr_next)
        xr_cur = [t_[:] for t_ in xr_next]

    emit_trend(NCHUNK - 2, xr_cur)
    emit_trend(NCHUNK - 1, xr_cur)


def build_nc(reps=1, acc_bufs=6, merge_tps=True, mmdt=_f32r):
    nc = bacc.Bacc("TRN2", target_bir_lowering=False, debug=False)
    xr_d = nc.dram_tensor("xr", [4, 128, 4, RPC], mmdt, kind="ExternalInput")
    e2l_d = nc.dram_tensor("e2l", [S, 128, 2048], mmdt, kind="ExternalInput")
    e2rr_d = nc.dram_tensor("e2rr", [S, 128, 1920], mmdt, kind="ExternalInput")
    tb_d = nc.dram_tensor("tb", [128, 3, 128], mmdt, kind="ExternalInput")
    out_d = nc.dram_tensor("out", [128, NCHUNK, RPC], _f32, kind="ExternalOutput")

    with tile.TileContext(nc) as tc:
        with (
            tc.tile_pool(name="const", bufs=1) as constp,
            tc.tile_pool(name="xrp", bufs=2) as xrp,
            tc.tile_pool(name="corrp", bufs=1) as corrp,
            tc.tile_pool(name="work", bufs=4) as workp,
            tc.tile_pool(name="psum_a", bufs=acc_bufs, space="PSUM") as psum_a,
            tc.tile_pool(name="psum_t", bufs=2, space="PSUM") as psum_t,
        ):
            pools = (constp, xrp, corrp, workp, psum_a,
                     psum_a if merge_tps else psum_t)
            if reps == 1:
                _emit_body(nc, pools, xr_d, e2l_d, e2rr_d, tb_d, out_d, mmdt)
            else:
                with tc.For_i(0, reps, 1, staggered_reset=True,
                              hint_engines=(mybir.EngineType.PE,)):
                    _emit_body(nc, pools, xr_d, e2l_d, e2rr_d, tb_d, out_d, mmdt)
    nc.compile()
    return nc


_NC_CACHE = {}


def _get_nc(reps=1):
    if reps not in _NC_CACHE:
        _NC_CACHE[reps] = build_nc(reps)
    return _NC_CACHE[reps]


def make_in_maps(x, diagonals, np_dt=np.float32):
    c = _circ_cols(diagonals)
    e2r = c[:, _E2R_IDX]  # [S, 128, 4096]
    e2l = np.ascontiguousarray(e2r[:, :, 127:2175]).astype(np_dt)
    e2rr = np.ascontiguousarray(e2r[:, :, 2175:4095]).astype(np_dt)
    xT = np.asarray(x, dtype=np.float32).reshape(ROWS, L).T  # [L, ROWS] view
    in_maps = []
    for i in range(NCORES):
        xs = np.ascontiguousarray(xT[:, i * RPC : (i + 1) * RPC])
        xs = xs.reshape(NCHUNK, 128, RPC).transpose(1, 0, 2)  # [128, 16, RPC]
        xs = np.ascontiguousarray(xs.reshape(128, 4, 4, RPC).transpose(1, 0, 2, 3))
        in_maps.append({"xr": xs.astype(np_dt), "e2l": e2l, "e2rr": e2rr, "tb": _TBAND.astype(np_dt)})
    return in_maps


def gather_out(results):
    parts = []
    for r in results:
        o = r["out"]  # [128, NCHUNK, RPC]
        parts.append(np.ascontiguousarray(o.transpose(1, 0, 2)).reshape(L, RPC))
    outT = np.concatenate(parts, axis=1)  # [L, ROWS]
    return np.ascontiguousarray(outT.T).reshape(B, C, L).astype(np.float32)


def kernel(x, diagonals):
    x = np.asarray(x, dtype=np.float32)
    assert x.shape == (B, C, L) and np.asarray(diagonals).shape == (S, L)
    nc = _get_nc(1)
    in_maps = make_in_maps(x, diagonals)
    last_err = None
    for attempt in range(3):
        try:
            res = run_bass_kernel_spmd(nc, in_maps, core_ids=list(range(NCORES)))
            return gather_out(res.results)
        except Exception as ex:  # transient device errors (e.g. NRT_EXEC_UNIT_UNRECOVERABLE)
            last_err = ex
            import time as _time

            _time.sleep(2.0 * (attempt + 1))
    raise last_err
